# revision 2
# baseline (speedup 1.0000x reference)
"""InterleavedHeadAttention Trainium2 kernel, v2.

Sharding (8 cores): core c handles batch b = c//4 and 4 output heads
g = c%4 (heads 4g..4g+3).  alpha head-mixing is folded into QKV
projection weights on the host.  The pseudo-head merge uses (p, n)
flat ordering (attention is permutation invariant; the token-causal
mask depends only on n).

v2 vs baseline:
- All folded weights are baked into the NEFF as fp8 constants (full 16
  heads); each core DMAs its head-group slice selected at runtime via
  partition_id() -> DynSlice.  Per-exec external I/O drops from 11MB to
  ~3.3MB (x8 input 1.25MB fp8 + output 2MB bf16).
- QKV and output projections run as fp8 DoubleRow matmuls (2 contraction
  rows/cycle); projection biases ride an extra contraction pair.
- Attention q/k/v are dequantized to bf16 on PSUM->SBUF copy (DVE/Pool
  tensor_scalar with per-call scale shipped via a tiny input).
- exp is batched over both pq scoreboards ([128,2,512] two-bank PSUM
  tiles); attention output is stored fp8*SO for the DoubleRow o_proj.
- Output partial is bf16; host accumulates in f32 and adds bo.
"""
import hashlib
import numpy as np
import ml_dtypes

import concourse.bacc as bacc
import concourse.bass as bass
import concourse.tile as tile
import concourse.mybir as mybir
from concourse.bass_utils import run_bass_kernel_spmd

B, S, HID, H, P = 2, 1024, 1024, 16, 2
D = HID // H          # 64
HL = 4                # heads per core
G = HL * P            # (h,p) groups per core = 8
HPD = HL * P * D      # 512 projection rows per core
HPD_ALL = H * P * D   # 2048
KT = HID // 128       # 8 contraction tiles
KP = KT // 2          # 4 DoubleRow contraction pairs
NT = S // 512         # 2 n windows
BF = mybir.dt.bfloat16
F8 = mybir.dt.float8e4
F32 = mybir.dt.float32
bf = ml_dtypes.bfloat16
f8 = np.dtype(mybir.dt.np(F8))
NCORES = 8
SX = 16.0             # hidden_states fp8 scale
SB0 = 16.0            # bias-pair x value
SO = 16.0             # attention-output fp8 scale

_cache = {}


def _build(consts, scales):
    """consts: dict with wq8/wk8/wv8 (128, 2*KP+2, HPD_ALL) f8,
    wo8 (128, H, HID) f8, tri (128,128) bf16.
    scales: dict sq/sk/sv (dequant mults for q/k/v psum) and so (oproj)."""
    nc = bacc.Bacc()
    x8 = nc.dram_tensor("x8", (128, 2 * KP + 2, S), F8, kind="ExternalInput")
    out = nc.dram_tensor("o", (S, HID), BF, kind="ExternalOutput")
    wq_d = nc.inline_tensor(consts["wq8"], name="wq8")
    wk_d = nc.inline_tensor(consts["wk8"], name="wk8")
    wv_d = nc.inline_tensor(consts["wv8"], name="wv8")
    wo_d = nc.inline_tensor(consts["wo8"], name="wo8")
    tri_d = nc.inline_tensor(consts["tri"], name="tri")
    sq, sk, sv, so = scales["sq"], scales["sk"], scales["sv"], scales["so"]

    with tile.TileContext(nc) as tc:
        with tc.tile_pool(name="persist", bufs=1) as pp, \
             tc.tile_pool(name="ppool", bufs=4) as ppl, \
             tc.tile_pool(name="small", bufs=4) as sml, \
             tc.tile_pool(name="osb", bufs=3) as osb, \
             tc.tile_pool(name="ps", bufs=2, space=bass.MemorySpace.PSUM) as ps, \
             tc.tile_pool(name="wide", bufs=2, space=bass.MemorySpace.PSUM) as wps, \
             tc.tile_pool(name="psav", bufs=2, space=bass.MemorySpace.PSUM) as psav:

            pid = nc.partition_id()
            g = pid % 4

            tri_sb = pp.tile([128, 128], BF, tag="tri", name="tri")
            nc.scalar.dma_start(tri_sb[:], tri_d[:])

            # input DMAs: x8 + first-head weights first on the SP HWDGE
            # queue so the first projection can start ASAP; the rest on the
            # Activation HWDGE queue in parallel.
            w_sb = {}
            for nm, dram in (("q", wq_d), ("k", wk_d), ("v", wv_d)):
                w_sb[nm] = pp.tile([128, 2 * KP + 2, HPD], F8,
                                   tag=f"w{nm}", name=f"w{nm}sb")
            x_sb = pp.tile([128, 2 * KP + 2, S], F8, tag="x8", name="x8sb")

            def dma_x(half):
                nc.sync.dma_start(
                    x_sb[:, :, half * 512:(half + 1) * 512],
                    x8[:, :, half * 512:(half + 1) * 512])

            def dma_w(nm, dram, mt):
                nc.sync.dma_start(
                    w_sb[nm][:, :, mt * 128:(mt + 1) * 128],
                    dram[:, :, bass.ds(g * HPD + mt * 128, 128)])

            dma_x(0)
            dma_w("q", wq_d, 0)
            dma_w("k", wk_d, 0)
            dma_x(1)
            for mt in range(1, HL):
                dma_w("q", wq_d, mt)
                dma_w("k", wk_d, mt)
            nc.scalar.dma_start(w_sb["v"][:], wv_d[:, :, bass.ds(g * HPD, HPD)])
            wo_sb = pp.tile([128, HL, HID], F8, tag="wo", name="wosb")
            nc.scalar.dma_start(wo_sb[:], wo_d[:, bass.ds(g * HL, HL), :])

            # ---- Q/K transposed projections: (hpd=128/head, n) ----
            qt_sb = [pp.tile([128, S], BF, tag=f"qt{h}", name=f"qt{h}") for h in range(HL)]
            kt_sb = [pp.tile([128, S], BF, tag=f"kt{h}", name=f"kt{h}") for h in range(HL)]
            kt2_sb = [pp.tile([128, S], BF, tag=f"kt2{h}", name=f"kt2{h}") for h in range(HL)]
            vaug = [pp.tile([128, G, 65], BF, tag=f"va{j}", name=f"va{j}")
                    for j in range(S // 128)]
            ot2 = pp.tile([128, HL, S], F8, tag="ot2", name="ot2")

            def proj_qk(nm, mt, nt):
                acc = ps.tile([128, 512], F32, tag="mm", name="mm")
                nsl = slice(nt * 512, (nt + 1) * 512)
                msl = slice(mt * 128, (mt + 1) * 128)
                for kk in range(KP + 1):
                    nc.tensor.matmul(
                        acc[:], w_sb[nm][:, 2 * kk:2 * kk + 2, msl],
                        x_sb[:, 2 * kk:2 * kk + 2, nsl],
                        start=(kk == 0), stop=(kk == KP),
                        perf_mode=mybir.MatmulPerfMode.DoubleRow)
                sc = sq if nm == "q" else sk
                if nm == "q":
                    nc.vector.tensor_scalar(
                        qt_sb[mt][:, nsl], acc[:], sc, None, mybir.AluOpType.mult)
                else:
                    nc.vector.tensor_scalar(
                        kt_sb[mt][:, nsl], acc[:], sc, None, mybir.AluOpType.mult)
                    # swapped-half copy for the pq=1 score tile, SBUF->SBUF
                    # on the Pool engine (gpsimd cannot touch PSUM)
                    nc.gpsimd.tensor_copy(
                        kt2_sb[mt][0:64, nsl], kt_sb[mt][64:128, nsl])
                    nc.gpsimd.tensor_copy(
                        kt2_sb[mt][64:128, nsl], kt_sb[mt][0:64, nsl])

            def proj_v(jt):
                v3 = vaug[jt]
                # ones column holds 1/SO so the softmax reciprocal directly
                # yields SO/den (folds the fp8 ot2 scale in for free)
                nc.gpsimd.memset(v3[:, :, 64:65], 1.0 / SO)
                acc = ps.tile([128, 512], F32, tag="mm", name="mm")
                jsl = slice(jt * 128, (jt + 1) * 128)
                for kk in range(KP + 1):
                    nc.tensor.matmul(
                        acc[:], x_sb[:, 2 * kk:2 * kk + 2, jsl],
                        w_sb["v"][:, 2 * kk:2 * kk + 2, :],
                        start=(kk == 0), stop=(kk == KP),
                        perf_mode=mybir.MatmulPerfMode.DoubleRow)
                nc.vector.tensor_scalar(
                    v3[:, :, 0:64], acc[:].rearrange("p (g e) -> p g e", e=64),
                    sv, None, mybir.AluOpType.mult)

            def attention(h, In, inject=None):
                    avp = [psav.tile([65, 512], F32, tag="av", name="av")
                           for _ in range(2)]
                    units = [(Jn, pk) for Jn in range(4 * In + 4)
                             for pk in range(2)]
                    pts = {}

                    def scores(u):
                        Jn, pk = u
                        FF = 128 * (Jn - 4 * In)
                        part = FF >= 0
                        c0 = FF if part else 0
                        jsl = slice(Jn * 128, (Jn + 1) * 128)
                        isl = slice(In * 512 + c0, (In + 1) * 512)
                        wide = wps.tile([128, 2, 512], F32, tag="sc", name="sc")
                        lhsA = (kt_sb[h] if pk == 0 else kt2_sb[h])
                        lhsB = (kt2_sb[h] if pk == 0 else kt_sb[h])
                        # wide[:, pq, :] -> scores (keys of pk) x (q of pq)
                        nc.tensor.matmul(
                            wide[:, 0, c0:512], lhsA[0:64, jsl],
                            qt_sb[h][0:64, isl], start=True, stop=True)
                        nc.tensor.matmul(
                            wide[:, 1, c0:512], lhsB[64:128, jsl],
                            qt_sb[h][64:128, isl], start=True, stop=True)
                        pt = ppl.tile([128, 2, 512], BF, tag="p", name="p")
                        nc.scalar.activation(
                            pt[:, :, c0:512], wide[:, :, c0:512],
                            mybir.ActivationFunctionType.Exp, scale=0.125)
                        if part:
                            tri_bc = tri_sb[:].unsqueeze(1).to_broadcast(
                                (128, 2, 128))
                            nc.vector.tensor_mul(
                                pt[:, :, c0:c0 + 128],
                                pt[:, :, c0:c0 + 128], tri_bc)
                        pts[u] = (pt, c0)

                    def av(u):
                        Jn, pk = u
                        pt, c0 = pts.pop(u)
                        gi = h * 2 + pk
                        for pq in range(2):
                            nc.tensor.matmul(
                                avp[pq][:, c0:512], vaug[Jn][:, gi, :],
                                pt[:, pq, c0:512],
                                start=(Jn == 0 and pk == 0),
                                stop=(Jn == 4 * In + 3 and pk == 1))

                    # software pipeline: scores(u+1) before av(u) so the PE
                    # in-order queue never stalls on the exp of unit u
                    scores(units[0])
                    for i in range(1, len(units)):
                        if inject:
                            inject(i)
                        scores(units[i])
                        av(units[i - 1])
                    av(units[-1])
                    for pq in range(2):
                        recip = sml.tile([1, 512], BF, tag="recip", name="recip")
                        with nc.allow_low_precision(reason="softmax recip bf16"):
                            # row 64 is den/SO, so recip = SO/den
                            nc.vector.reciprocal(recip[:], avp[pq][64:65, :])
                        bcs = sml.tile([64, 512], BF, tag="bcs", name="bcs")
                        nc.gpsimd.partition_broadcast(bcs[:], recip[:])
                        with nc.allow_low_precision(reason="fp8 attn out"):
                            nc.vector.tensor_mul(
                                ot2[pq * 64:(pq + 1) * 64, h,
                                    In * 512:(In + 1) * 512],
                                avp[pq][0:64, :], bcs[:])

            def oproj(mt):
                for jt in range(HID // 512):
                    op = ps.tile([128, 512], F32, tag="mm", name="mm")
                    for hh in range(HL // 2):
                        nc.tensor.matmul(
                            op[:], ot2[:, 2 * hh:2 * hh + 2,
                                       mt * 128:(mt + 1) * 128],
                            wo_sb[:, 2 * hh:2 * hh + 2,
                                  jt * 512:(jt + 1) * 512],
                            start=(hh == 0), stop=(hh == HL // 2 - 1),
                            perf_mode=mybir.MatmulPerfMode.DoubleRow)
                    ob = osb.tile([128, 512], BF, tag="ob", name="ob")
                    nc.vector.tensor_scalar(ob[:], op[:], so, None,
                                            mybir.AluOpType.mult)
                    nc.gpsimd.dma_start(
                        out[mt * 128:(mt + 1) * 128, jt * 512:(jt + 1) * 512],
                        ob[:])

            # emit: QK of h0 first so Act starts early, V next (needed by
            # first AV).  During the In=0 sweep, inject later heads' QK
            # projection units between attention blocks so the PE never
            # starves the Act pipeline; oproj of the first n-half overlaps
            # the In=1 sweep.
            for nt in range(NT):
                proj_qk("q", 0, nt)
                proj_qk("k", 0, nt)
            for jt in range(S // 128):
                proj_v(jt)
            pending = [(nm, h, nt) for h in range(1, HL)
                       for nt in range(NT) for nm in ("q", "k")]

            def inject(_):
                if pending:
                    nm, h, nt = pending.pop(0)
                    proj_qk(nm, h, nt)

            for h in range(HL):
                attention(h, 0, inject=inject)
            while pending:
                inject(0)
            for h in range(HL):
                attention(h, 1)
                if h == 0:
                    # n-window 0 of ot2 is complete for all heads
                    for mt in range(4):
                        oproj(mt)
            for mt in range(4, 8):
                oproj(mt)
    nc.compile()
    return nc


def _fold(inputs):
    """Host-side weight folding -> per-tensor-scaled fp8 consts + scales."""
    consts, scales = {}, {}
    sw = {}
    for nm in ("q", "k", "v"):
        W = np.asarray(inputs[f"W{nm}"], np.float32)
        bb = np.asarray(inputs[f"b{nm}"], np.float32)
        al = np.asarray(inputs[f"alpha_{nm}"], np.float32)
        We = np.einsum("mhp,mdc->hpdc", al, W.reshape(H, D, HID))
        We = We.reshape(HPD_ALL, HID)            # (m, c)
        be = np.einsum("mhp,md->hpd", al, bb.reshape(H, D)).reshape(HPD_ALL)
        s = 128.0 / max(np.abs(We).max(), 1e-30)
        sw[nm] = s
        w8 = np.zeros((128, 2 * KP + 2, HPD_ALL), f8)
        wt = (We.T * s).reshape(KT, 128, HPD_ALL)    # (k, c, m)
        w8[:, 0:KT, :] = wt.transpose(1, 0, 2).astype(f8)
        w8[0, KT, :] = (be * SX * s / SB0).astype(f8)   # bias pair, tile KT
        consts[f"w{nm}8"] = w8
        scales[f"s{nm}"] = float(1.0 / (SX * s))
    Wo = np.asarray(inputs["Wo"], np.float32)
    col = np.asarray(inputs["collapse"], np.float32)
    Woe = np.einsum("hp,jhd->hpdj", col, Wo.reshape(HID, H, D))  # (H,P,D,HID)
    swo = 128.0 / max(np.abs(Woe).max(), 1e-30)
    consts["wo8"] = np.ascontiguousarray(
        (Woe.reshape(H, P * D, HID) * swo).transpose(1, 0, 2)).astype(f8)
    scales["so"] = float(1.0 / (SO * swo))
    consts["tri"] = np.triu(np.ones((128, 128), np.float32)).astype(bf)
    return consts, scales


def _prep_x(inputs):
    """Per-core x8 input: (128, 2*KP+2, S) fp8."""
    maps = []
    for c in range(NCORES):
        b = c // 4
        hs = np.asarray(inputs["hidden_states"], np.float32)[b]  # (S, HID)
        x8 = np.zeros((128, 2 * KP + 2, S), f8)
        xt = (hs.T * SX).reshape(KT, 128, S)         # (k, c, n)
        x8[:, 0:KT, :] = xt.transpose(1, 0, 2).astype(f8)
        x8[0, KT, :] = np.asarray(SB0, f8)           # bias pair
        maps.append({"x8": x8})
    return maps


def _key(inputs):
    hsh = hashlib.sha256()
    for nm in ("Wq", "bq", "Wk", "bk", "Wv", "bv", "Wo", "bo",
               "alpha_q", "alpha_k", "alpha_v", "collapse"):
        hsh.update(np.ascontiguousarray(np.asarray(inputs[nm])).tobytes())
    return hsh.hexdigest()


def kernel(**inputs):
    key = _key(inputs)
    if key not in _cache:
        consts, scales = _fold(inputs)
        _cache.clear()
        _cache[key] = _build(consts, scales)
    nc = _cache[key]
    maps = _prep_x(inputs)
    res = run_bass_kernel_spmd(nc, maps, core_ids=list(range(NCORES)))
    bo = np.asarray(inputs["bo"], np.float32)
    out = np.zeros((B, S, HID), np.float32)
    for c in range(NCORES):
        out[c // 4] += np.asarray(res.results[c]["o"], np.float32)
    out += bo
    return out


# revision 3
# speedup vs baseline: 1.5954x; 1.5954x over previous
"""InterleavedHeadAttention Trainium2 kernel, v2.

Sharding (8 cores): core c handles batch b = c//4 and 4 output heads
g = c%4 (heads 4g..4g+3).  alpha head-mixing is folded into QKV
projection weights on the host.  The pseudo-head merge uses (p, n)
flat ordering (attention is permutation invariant; the token-causal
mask depends only on n).

v2 vs baseline:
- All folded weights are baked into the NEFF as fp8 constants (full 16
  heads); each core DMAs its head-group slice selected at runtime via
  partition_id() -> DynSlice.  Per-exec external I/O drops from 11MB to
  ~3.3MB (x8 input 1.25MB fp8 + output 2MB bf16).
- QKV and output projections run as fp8 DoubleRow matmuls (2 contraction
  rows/cycle); projection biases ride an extra contraction pair.
- Attention q/k/v are dequantized to bf16 on PSUM->SBUF copy (DVE/Pool
  tensor_scalar with per-call scale shipped via a tiny input).
- exp is batched over both pq scoreboards ([128,2,512] two-bank PSUM
  tiles); attention output is stored fp8*SO for the DoubleRow o_proj.
- Output partial is bf16; host accumulates in f32 and adds bo.
"""
import hashlib
import numpy as np
import ml_dtypes

import concourse.bacc as bacc
import concourse.bass as bass
import concourse.tile as tile
import concourse.mybir as mybir
from concourse.bass_utils import run_bass_kernel_spmd

B, S, HID, H, P = 2, 1024, 1024, 16, 2
D = HID // H          # 64
HL = 4                # heads per core
G = HL * P            # (h,p) groups per core = 8
HPD = HL * P * D      # 512 projection rows per core
HPD_ALL = H * P * D   # 2048
KT = HID // 128       # 8 contraction tiles
KP = KT // 2          # 4 DoubleRow contraction pairs
NT = S // 512         # 2 n windows
BF = mybir.dt.bfloat16
F8 = mybir.dt.float8e4
F32 = mybir.dt.float32
bf = ml_dtypes.bfloat16
f8 = np.dtype(mybir.dt.np(F8))
NCORES = 8
SX = 16.0             # hidden_states fp8 scale
SB0 = 16.0            # bias-pair x value
SO = 16.0             # attention-output fp8 scale

_cache = {}


def _build(consts, scales):
    """consts: dict with wq8/wk8/wv8 (128, 2*KP+2, HPD_ALL) f8,
    wo8 (128, H, HID) f8, tri (128,128) bf16.
    scales: dict sq/sk/sv (dequant mults for q/k/v psum) and so (oproj)."""
    nc = bacc.Bacc()
    x8 = nc.dram_tensor("x8", (128, KT, S), F8, kind="ExternalInput")
    out = nc.dram_tensor("o", (S, HID), BF, kind="ExternalOutput")
    wq_d = nc.inline_tensor(consts["wq8"], name="wq8")
    wk_d = nc.inline_tensor(consts["wk8"], name="wk8")
    wv_d = nc.inline_tensor(consts["wv8"], name="wv8")
    wo_d = nc.inline_tensor(consts["wo8"], name="wo8")
    tri_d = nc.inline_tensor(consts["tri"], name="tri")
    bias_d = nc.inline_tensor(consts["biasT"], name="biasT")
    sq, sk, sv, so = scales["sq"], scales["sk"], scales["sv"], scales["so"]

    with tile.TileContext(nc) as tc:
        with tc.tile_pool(name="persist", bufs=1) as pp, \
             tc.tile_pool(name="ppool", bufs=4) as ppl, \
             tc.tile_pool(name="small", bufs=4) as sml, \
             tc.tile_pool(name="osb", bufs=3) as osb, \
             tc.tile_pool(name="ps", bufs=2, space=bass.MemorySpace.PSUM) as ps, \
             tc.tile_pool(name="wide", bufs=2, space=bass.MemorySpace.PSUM) as wps, \
             tc.tile_pool(name="psav", bufs=2, space=bass.MemorySpace.PSUM) as psav:

            pid = nc.partition_id()
            g = pid % 4

            tri_sb = pp.tile([128, 128], BF, tag="tri", name="tri")
            nc.scalar.dma_start(tri_sb[:], tri_d[:])

            # input DMAs: x8 + first-head weights first on the SP HWDGE
            # queue so the first projection can start ASAP; the rest on the
            # Activation HWDGE queue in parallel.
            w_sb = {}
            for nm, dram in (("q", wq_d), ("k", wk_d), ("v", wv_d)):
                w_sb[nm] = pp.tile([128, KT, HPD], F8,
                                   tag=f"w{nm}", name=f"w{nm}sb")
            x_sb = pp.tile([128, KT, S], F8, tag="x8", name="x8sb")
            bias_sb = pp.tile([128, 2, HL], F32, tag="bias", name="biassb")
            nc.scalar.dma_start(bias_sb[:], bias_d[:, :, bass.ds(g * HL, HL)])

            def dma_x(half):
                nc.sync.dma_start(
                    x_sb[:, :, half * 512:(half + 1) * 512],
                    x8[:, :, half * 512:(half + 1) * 512])

            def dma_w(nm, dram, mt):
                nc.sync.dma_start(
                    w_sb[nm][:, :, mt * 128:(mt + 1) * 128],
                    dram[:, :, bass.ds(g * HPD + mt * 128, 128)])

            dma_x(0)
            dma_w("q", wq_d, 0)
            dma_w("k", wk_d, 0)
            dma_x(1)
            for mt in range(1, HL):
                dma_w("q", wq_d, mt)
                dma_w("k", wk_d, mt)
            nc.scalar.dma_start(w_sb["v"][:], wv_d[:, :, bass.ds(g * HPD, HPD)])
            wo_sb = pp.tile([128, HL, HID], F8, tag="wo", name="wosb")
            nc.scalar.dma_start(wo_sb[:], wo_d[:, bass.ds(g * HL, HL), :])

            # ---- Q/K transposed projections: (hpd=128/head, n) ----
            qt_sb = [pp.tile([128, S], BF, tag=f"qt{h}", name=f"qt{h}") for h in range(HL)]
            kt_sb = [pp.tile([128, S], BF, tag=f"kt{h}", name=f"kt{h}") for h in range(HL)]
            kt2_sb = [pp.tile([128, S], BF, tag=f"kt2{h}", name=f"kt2{h}") for h in range(HL)]
            vaug = [pp.tile([128, G, 65], BF, tag=f"va{j}", name=f"va{j}")
                    for j in range(S // 128)]
            ot2 = pp.tile([128, HL, S], F8, tag="ot2", name="ot2")

            def proj_qk(nm, mt, nt):
                acc = ps.tile([128, 512], F32, tag="mm", name="mm")
                nsl = slice(nt * 512, (nt + 1) * 512)
                msl = slice(mt * 128, (mt + 1) * 128)
                for kk in range(KP):
                    nc.tensor.matmul(
                        acc[:], w_sb[nm][:, 2 * kk:2 * kk + 2, msl],
                        x_sb[:, 2 * kk:2 * kk + 2, nsl],
                        start=(kk == 0), stop=(kk == KP - 1),
                        perf_mode=mybir.MatmulPerfMode.DoubleRow)
                sc = sq if nm == "q" else sk
                bia = bias_sb[:, 0 if nm == "q" else 1, mt:mt + 1]
                if nm == "q":
                    nc.vector.tensor_scalar(
                        qt_sb[mt][:, nsl], acc[:], sc, bia,
                        mybir.AluOpType.mult, mybir.AluOpType.add)
                else:
                    nc.vector.tensor_scalar(
                        kt_sb[mt][:, nsl], acc[:], sc, bia,
                        mybir.AluOpType.mult, mybir.AluOpType.add)
                    # swapped-half copy for the pq=1 score tile, SBUF->SBUF
                    # on the Pool engine (gpsimd cannot touch PSUM)
                    nc.gpsimd.tensor_copy(
                        kt2_sb[mt][0:64, nsl], kt_sb[mt][64:128, nsl])
                    nc.gpsimd.tensor_copy(
                        kt2_sb[mt][64:128, nsl], kt_sb[mt][0:64, nsl])

            def proj_v(jt):
                v3 = vaug[jt]
                # ones column holds 1/SO so the softmax reciprocal directly
                # yields SO/den (folds the fp8 ot2 scale in for free)
                nc.gpsimd.memset(v3[:, :, 64:65], 1.0 / SO)
                acc = ps.tile([128, 512], F32, tag="mm", name="mm")
                jsl = slice(jt * 128, (jt + 1) * 128)
                for kk in range(KP):
                    nc.tensor.matmul(
                        acc[:], x_sb[:, 2 * kk:2 * kk + 2, jsl],
                        w_sb["v"][:, 2 * kk:2 * kk + 2, :],
                        start=(kk == 0), stop=(kk == KP - 1),
                        perf_mode=mybir.MatmulPerfMode.DoubleRow)
                nc.vector.tensor_scalar(
                    v3[:, :, 0:64], acc[:].rearrange("p (g e) -> p g e", e=64),
                    sv, None, mybir.AluOpType.mult)

            def attention(h, In, inject=None):
                    avp = [psav.tile([65, 512], F32, tag="av", name="av")
                           for _ in range(2)]
                    units = [(Jn, pk) for Jn in range(4 * In + 4)
                             for pk in range(2)]
                    pts = {}

                    def scores(u):
                        Jn, pk = u
                        FF = 128 * (Jn - 4 * In)
                        part = FF >= 0
                        c0 = FF if part else 0
                        jsl = slice(Jn * 128, (Jn + 1) * 128)
                        isl = slice(In * 512 + c0, (In + 1) * 512)
                        wide = wps.tile([128, 2, 512], F32, tag="sc", name="sc")
                        lhsA = (kt_sb[h] if pk == 0 else kt2_sb[h])
                        lhsB = (kt2_sb[h] if pk == 0 else kt_sb[h])
                        # wide[:, pq, :] -> scores (keys of pk) x (q of pq)
                        nc.tensor.matmul(
                            wide[:, 0, c0:512], lhsA[0:64, jsl],
                            qt_sb[h][0:64, isl], start=True, stop=True)
                        nc.tensor.matmul(
                            wide[:, 1, c0:512], lhsB[64:128, jsl],
                            qt_sb[h][64:128, isl], start=True, stop=True)
                        pt = ppl.tile([128, 2, 512], BF, tag="p", name="p")
                        nc.scalar.activation(
                            pt[:, :, c0:512], wide[:, :, c0:512],
                            mybir.ActivationFunctionType.Exp, scale=0.125)
                        if part:
                            tri_bc = tri_sb[:].unsqueeze(1).to_broadcast(
                                (128, 2, 128))
                            nc.vector.tensor_mul(
                                pt[:, :, c0:c0 + 128],
                                pt[:, :, c0:c0 + 128], tri_bc)
                        pts[u] = (pt, c0)

                    def av(u):
                        Jn, pk = u
                        pt, c0 = pts.pop(u)
                        gi = h * 2 + pk
                        for pq in range(2):
                            nc.tensor.matmul(
                                avp[pq][:, c0:512], vaug[Jn][:, gi, :],
                                pt[:, pq, c0:512],
                                start=(Jn == 0 and pk == 0),
                                stop=(Jn == 4 * In + 3 and pk == 1))

                    # software pipeline: scores(u+1) before av(u) so the PE
                    # in-order queue never stalls on the exp of unit u
                    scores(units[0])
                    for i in range(1, len(units)):
                        if inject:
                            inject(i)
                        scores(units[i])
                        av(units[i - 1])
                    av(units[-1])
                    for pq in range(2):
                        recip = sml.tile([1, 512], BF, tag="recip", name="recip")
                        with nc.allow_low_precision(reason="softmax recip bf16"):
                            # row 64 is den/SO, so recip = SO/den
                            nc.vector.reciprocal(recip[:], avp[pq][64:65, :])
                        bcs = sml.tile([64, 512], BF, tag="bcs", name="bcs")
                        nc.gpsimd.partition_broadcast(bcs[:], recip[:])
                        with nc.allow_low_precision(reason="fp8 attn out"):
                            nc.vector.tensor_mul(
                                ot2[pq * 64:(pq + 1) * 64, h,
                                    In * 512:(In + 1) * 512],
                                avp[pq][0:64, :], bcs[:])

            def oproj(mt):
                for jt in range(HID // 512):
                    op = ps.tile([128, 512], F32, tag="mm", name="mm")
                    for hh in range(HL // 2):
                        nc.tensor.matmul(
                            op[:], ot2[:, 2 * hh:2 * hh + 2,
                                       mt * 128:(mt + 1) * 128],
                            wo_sb[:, 2 * hh:2 * hh + 2,
                                  jt * 512:(jt + 1) * 512],
                            start=(hh == 0), stop=(hh == HL // 2 - 1),
                            perf_mode=mybir.MatmulPerfMode.DoubleRow)
                    ob = osb.tile([128, 512], BF, tag="ob", name="ob")
                    nc.vector.tensor_scalar(ob[:], op[:], so, None,
                                            mybir.AluOpType.mult)
                    nc.gpsimd.dma_start(
                        out[mt * 128:(mt + 1) * 128, jt * 512:(jt + 1) * 512],
                        ob[:])

            # emit: QK of h0 first so Act starts early, V next (needed by
            # first AV).  During the In=0 sweep, inject later heads' QK
            # projection units between attention blocks so the PE never
            # starves the Act pipeline; oproj of the first n-half overlaps
            # the In=1 sweep.
            for nt in range(NT):
                proj_qk("q", 0, nt)
                proj_qk("k", 0, nt)
            for jt in range(S // 128):
                proj_v(jt)
            pending = [(nm, h, nt) for h in range(1, HL)
                       for nt in range(NT) for nm in ("q", "k")]

            def inject(_):
                if pending:
                    nm, h, nt = pending.pop(0)
                    proj_qk(nm, h, nt)

            for h in range(HL):
                attention(h, 0, inject=inject)
            while pending:
                inject(0)
            for h in range(HL):
                attention(h, 1)
                if h == 0:
                    # n-window 0 of ot2 is complete for all heads
                    for mt in range(4):
                        oproj(mt)
            for mt in range(4, 8):
                oproj(mt)
    nc.compile()
    return nc


def _fold(inputs):
    """Host-side weight folding -> per-tensor-scaled fp8 consts + scales.

    Also returns bvwo: the o-projection of the (constant) V bias.  Softmax
    weights sum to 1, so av = sum(p*v)/den + bv and the bv term contributes
    a constant row sum_hpd bv[h,pd]*Woe[h,pd,:] added host-side with bo.
    """
    consts, scales = {}, {}
    bias_rows = {}
    for nm in ("q", "k", "v"):
        W = np.asarray(inputs[f"W{nm}"], np.float32)
        bb = np.asarray(inputs[f"b{nm}"], np.float32)
        al = np.asarray(inputs[f"alpha_{nm}"], np.float32)
        We = np.einsum("mhp,mdc->hpdc", al, W.reshape(H, D, HID))
        We = We.reshape(HPD_ALL, HID)            # (m, c)
        be = np.einsum("mhp,md->hpd", al, bb.reshape(H, D)).reshape(HPD_ALL)
        bias_rows[nm] = be
        s = 128.0 / max(np.abs(We).max(), 1e-30)
        wt = (We.T * s).reshape(KT, 128, HPD_ALL)    # (k, c, m)
        consts[f"w{nm}8"] = np.ascontiguousarray(
            wt.transpose(1, 0, 2)).astype(f8)
        scales[f"s{nm}"] = float(1.0 / (SX * s))
    # biasT const: [p, {q,k}, mt_global] with m = mt_global*128 + p
    biasT = np.zeros((128, 2, H), np.float32)
    for i, nm in enumerate(("q", "k")):
        biasT[:, i, :] = bias_rows[nm].reshape(H, 128).T
    consts["biasT"] = biasT
    Wo = np.asarray(inputs["Wo"], np.float32)
    col = np.asarray(inputs["collapse"], np.float32)
    Woe = np.einsum("hp,jhd->hpdj", col, Wo.reshape(HID, H, D))  # (H,P,D,HID)
    swo = 128.0 / max(np.abs(Woe).max(), 1e-30)
    consts["wo8"] = np.ascontiguousarray(
        (Woe.reshape(H, P * D, HID) * swo).transpose(1, 0, 2)).astype(f8)
    scales["so"] = float(1.0 / (SO * swo))
    consts["tri"] = np.triu(np.ones((128, 128), np.float32)).astype(bf)
    bvwo = np.einsum("m,mj->j", bias_rows["v"],
                     Woe.reshape(HPD_ALL, HID))
    return consts, scales, bvwo


def _prep_x(inputs):
    """Per-core x8 input: (128, KT, S) fp8."""
    maps = []
    x8b = []
    for b in range(B):
        hs = np.asarray(inputs["hidden_states"], np.float32)[b]  # (S, HID)
        xt = (hs.T * SX).reshape(KT, 128, S)         # (k, c, n)
        x8b.append(np.ascontiguousarray(xt.transpose(1, 0, 2)).astype(f8))
    for c in range(NCORES):
        maps.append({"x8": x8b[c // 4]})
    return maps


def _key(inputs):
    hsh = hashlib.sha256()
    for nm in ("Wq", "bq", "Wk", "bk", "Wv", "bv", "Wo", "bo",
               "alpha_q", "alpha_k", "alpha_v", "collapse"):
        hsh.update(np.ascontiguousarray(np.asarray(inputs[nm])).tobytes())
    return hsh.hexdigest()


def kernel(**inputs):
    key = _key(inputs)
    if key not in _cache:
        consts, scales, bvwo = _fold(inputs)
        _cache.clear()
        _cache[key] = (_build(consts, scales), bvwo)
    nc, bvwo = _cache[key]
    maps = _prep_x(inputs)
    res = run_bass_kernel_spmd(nc, maps, core_ids=list(range(NCORES)))
    bo = np.asarray(inputs["bo"], np.float32)
    out = np.zeros((B, S, HID), np.float32)
    for c in range(NCORES):
        out[c // 4] += np.asarray(res.results[c]["o"], np.float32)
    out += bo + bvwo
    return out
